# revision 13
# baseline (speedup 1.0000x reference)
"""Chamfer loss kernel for Trainium2 (8 NeuronCores, data-parallel over batch).

Math: for each batch, d2[m,n] = p2[m] + g2[n] - 2*dot(pred_m, gt_n). The
reference's gather+recompute equals min_n d2 (resp. min_m), so only row/col
mins are needed:
  loss = mean(relu(sqrt(rowmin d2 + EPS) - t)) + mean(relu(sqrt(colmin + EPS) - t))

Design (per core = 2 batches):
- The matmul computes -d2 DIRECTLY via an augmented K=20 contraction: row
  blocks of 5 per (direction, batch): A = [2*own; -own2; -1] vs
  B = [other; 1; other2]. Inactive blocks are zero in A, so one weight layout
  covers BOTH chamfer directions (fwd: pred->gt, bwd: gt->pred as a second,
  transposed pass) and both batches — colmin never needs a partition
  reduction, everything is a row-max of -d2.
- One hardware For_i over 128 m-tiles: stage weights [20,128] (ldweights
  can't take register offsets), 8 matmuls (float32r: 1 cycle/col vs 4 for
  plain fp32) into two [128,2048] PSUM tiles (4 banks each, double-buffered),
  and one DVE tensor_reduce(max) per PSUM tile straight from PSUM into
  fwd[:, i].
- reps (for the reps-delta timing harness) run as an OUTER hardware loop, so
  repeated invocations re-execute the same instruction stream the way real
  repeated calls would. Input tiles are double-buffered across reps so the
  next rep's DMA overlaps the current rep's compute.
- Host epilogue: sqrt/relu/mean over 8*2*8192 values (negligible).

float32r matmul numerics cost ~1.2e-3 relative error on the final loss
(tolerance 2e-2); exact-fp32 variant (mm_dtype="f32") measured 4.9e-7.
"""

import numpy as np

EPS = 1e-8
B, M, N = 16, 4096, 4096
NCORES = 8
B_LOC = B // NCORES  # batches per core
K = 20  # contraction: 2 dirs x 2 batches x 5 augmented rows
NT = 128  # m-tiles: 2 dirs x 2 batches x 32

_CACHE = {}


def build_nc(
    reps=1,
    mm_dtype="f32r",
    psum_split=2,
    reps_mode="loop",
    rep_dbuf=True,
    copy_eng="vector",
    mm_n=512,
    T=8,
    unroll=False,
):
    import concourse.bacc as bacc
    import concourse.mybir as mybir
    import concourse.tile as tile
    from concourse.bass import ds
    from contextlib import ExitStack

    f32 = mybir.dt.float32
    f32r = mybir.dt.float32r
    MAX = mybir.AluOpType.max
    Copy = mybir.ActivationFunctionType.Copy
    E = mybir.EngineType

    nc = bacc.Bacc("TRN2", target_bir_lowering=False, debug=False)
    a_in = nc.dram_tensor("a_in", [K, NT * 128], f32, kind="ExternalInput").ap()
    b_in = nc.dram_tensor("b_in", [K, 4096], f32, kind="ExternalInput").ap()
    if unroll:
        T = 1  # tile-per-column output layout, no inner loop
    assert NT % T == 0
    n_seg = psum_split * T
    n_iter = NT // T
    fwd_out = nc.dram_tensor(
        "fwd_out", [128, n_seg, n_iter], f32, kind="ExternalOutput"
    ).ap()

    hints = (E.PE, E.Activation, E.DVE, E.SP, E.Pool)
    seg = 4096 // psum_split
    with tile.TileContext(nc) as tc, ExitStack() as ctx:
        pool = ctx.enter_context(tc.tile_pool(name="sb", bufs=1))
        in_pool = ctx.enter_context(
            tc.tile_pool(name="inp", bufs=2 if rep_dbuf else 1)
        )
        ps_pool = ctx.enter_context(
            tc.tile_pool(name="ps", bufs=psum_split, space="PSUM")
        )
        wp = ctx.enter_context(tc.tile_pool(name="w", bufs=2))

        def rep_body():
            a_sb = in_pool.tile([K, NT * 128], f32, tag="a")
            b_sb = in_pool.tile([K, 4096], f32, tag="b")
            nc.sync.dma_start(out=a_sb, in_=a_in)
            nc.sync.dma_start(out=b_sb, in_=b_in)
            fwd = pool.tile([128, n_seg, n_iter], f32, tag="fwd")
            nc.vector.memset(fwd, 0.0)

            if mm_dtype == "f32r":
                # float32r operands must be produced by a rounding compute op,
                # not a DMA (BIR verifier requirement).
                b_mm = pool.tile([K, 4096], f32r, tag="br")
                nc.vector.tensor_copy(out=b_mm, in_=b_sb)
                w_dt = f32r
            else:
                b_mm = b_sb
                w_dt = f32

            if unroll:
                # Fully unrolled: static weight slices (no staging copy, no
                # loop control). One rounding copy of all weights up front.
                if mm_dtype == "f32r":
                    a_mm = pool.tile([K, NT * 128], f32r, tag="ar")
                    nc.vector.tensor_copy(out=a_mm, in_=a_sb)
                else:
                    a_mm = a_sb
                for g in range(NT):
                    for h in range(psum_split):
                        ps = ps_pool.tile([128, seg], f32, tag="ps")
                        for j in range(seg // mm_n):
                            n0 = h * seg + j * mm_n
                            nc.tensor.matmul(
                                ps[:, j * mm_n : (j + 1) * mm_n],
                                a_mm[:, g * 128 : (g + 1) * 128],
                                b_mm[:, n0 : n0 + mm_n],
                                start=True,
                                stop=True,
                            )
                        nc.vector.tensor_reduce(
                            out=fwd[:, h, g : g + 1],
                            in_=ps,
                            axis=mybir.AxisListType.X,
                            op=MAX,
                        )
                nc.sync.dma_start(out=fwd_out, in_=fwd)
                return

            with tc.For_i(0, n_iter, 1, hint_engines=hints) as i:
                wcur = wp.tile([K, T * 128], w_dt, tag="wc")
                if copy_eng == "scalar":
                    nc.scalar.activation(
                        out=wcur, in_=a_sb[:, ds(i * (T * 128), T * 128)], func=Copy
                    )
                else:
                    nc.vector.tensor_copy(
                        out=wcur, in_=a_sb[:, ds(i * (T * 128), T * 128)]
                    )
                for t in range(T):
                    for h in range(psum_split):
                        ps = ps_pool.tile([128, seg], f32, tag="ps")
                        for j in range(seg // mm_n):
                            n0 = h * seg + j * mm_n
                            nc.tensor.matmul(
                                ps[:, j * mm_n : (j + 1) * mm_n],
                                wcur[:, t * 128 : (t + 1) * 128],
                                b_mm[:, n0 : n0 + mm_n],
                                start=True,
                                stop=True,
                            )
                        nc.vector.tensor_reduce(
                            out=fwd[:, t * psum_split + h, ds(i, 1)],
                            in_=ps,
                            axis=mybir.AxisListType.X,
                            op=MAX,
                        )
            nc.sync.dma_start(out=fwd_out, in_=fwd)

        if reps_mode == "loop" and reps > 1:
            with tc.For_i(0, reps, 1, hint_engines=hints):
                rep_body()
        else:
            for _ in range(max(1, reps) if reps_mode != "loop" else 1):
                rep_body()
    nc.compile()
    return nc


def _host_prep(predict_pc_6, gt_pc_6):
    """Build per-core augmented operands A [NCORES, K, NT*128], Bm [NCORES, K, 4096].

    Row blocks (5 rows each): r = d*10 + bb*5 for direction d (0: pred->gt,
    1: gt->pred) and core-local batch bb. A column tile t in 0..127 maps to
    d = t//64, bb = (t//32)%2, m-tile = t%32; only the (d, bb) block rows are
    nonzero there, which also selects the direction/batch on the B side.
    """
    pred = np.ascontiguousarray(predict_pc_6[:, :3, :], dtype=np.float32)
    gt = np.ascontiguousarray(gt_pc_6[:, :3, :], dtype=np.float32)
    p2 = np.einsum("bdm,bdm->bm", pred, pred)
    g2 = np.einsum("bdm,bdm->bm", gt, gt)

    A = np.zeros((NCORES, K, NT * 128), np.float32)
    Bm = np.empty((NCORES, K, 4096), np.float32)
    for c in range(NCORES):
        for bb in range(B_LOC):
            gb = c * B_LOC + bb
            for d in range(2):
                own = pred[gb] if d == 0 else gt[gb]
                own2 = p2[gb] if d == 0 else g2[gb]
                oth = gt[gb] if d == 0 else pred[gb]
                oth2 = g2[gb] if d == 0 else p2[gb]
                r = d * 10 + bb * 5
                Bm[c, r : r + 3] = oth
                Bm[c, r + 3] = 1.0
                Bm[c, r + 4] = oth2
                c0 = (d * 64 + bb * 32) * 128
                A[c, r : r + 3, c0 : c0 + 4096] = 2.0 * own
                A[c, r + 3, c0 : c0 + 4096] = -own2
                A[c, r + 4, c0 : c0 + 4096] = -1.0
    return A, Bm


def _epilogue(results, thresh, psum_split=2, T=8):
    """fwd_out[p, s, i] with s = t*psum_split + h: segment h of m-tile i*T+t.
    Global tile g = i*T+t: direction = g//64, batch = (g//32)%2,
    point = (g%32)*128 + p."""
    n_iter = NT // T
    fwd_sum = 0.0
    bwd_sum = 0.0
    for r in results:
        v = r["fwd_out"].astype(np.float64).reshape(128, T, psum_split, n_iter)
        neg = v.max(axis=2)  # [128, T, n_iter] rowmax of -d2 per (t, i)
        neg = np.transpose(neg, (0, 2, 1)).reshape(128, NT)  # tile g = i*T+t
        dmin = np.maximum(-neg + EPS, 0.0)
        e = np.sqrt(dmin)
        relu = np.maximum(e - float(thresh), 0.0)
        fwd_sum += relu[:, 0:64].sum()
        bwd_sum += relu[:, 64:128].sum()
    return np.float32(fwd_sum / (B * M) + bwd_sum / (B * N))


def kernel(predict_pc_6, gt_pc_6, thresh):
    from concourse.bass_utils import run_bass_kernel_spmd

    predict_pc_6 = np.asarray(predict_pc_6)
    gt_pc_6 = np.asarray(gt_pc_6)
    thresh = np.float32(thresh)

    A, Bm = _host_prep(predict_pc_6, gt_pc_6)

    if "nc" not in _CACHE:
        _CACHE["nc"] = build_nc()
    nc = _CACHE["nc"]

    core_ids = list(range(NCORES))
    in_maps = [
        {"a_in": np.ascontiguousarray(A[i]), "b_in": np.ascontiguousarray(Bm[i])}
        for i in core_ids
    ]
    res = run_bass_kernel_spmd(nc, in_maps, core_ids)
    _CACHE["last_res"] = res
    return _epilogue([res.results[i] for i in core_ids], thresh)
